# revision 4
# baseline (speedup 1.0000x reference)
"""MCR2 loss kernel for 8 Trainium2 NeuronCores.

Class-sorted data-parallel sharding: the host permutes samples so each
class occupies a contiguous, zero-padded block of CAP rows (one-hot
masking then costs nothing on device).  Each core streams its 76800-row
shard of the permuted Z once as fp16.  Tensor work is batched four
128-sample tiles per matmul: stationary = moving = a [128, 128] column
block of four adjacent tiles, accumulated into one [128, 128] PSUM
region per 7680-row chunk.  The four diagonal 32x32 blocks of each
region are the per-tile Grams (off-diagonal cross blocks accumulate
into distinct PSUM addresses and are never read).  Every chunk lies
inside one class block, so chunk-Grams sum to class-Grams on the host,
where the 32x32 logdets are evaluated in float64.
"""

import sys

sys.path.insert(0, "/opt/trn_rl_repo")

import numpy as np

import concourse.bacc as bacc
import concourse.bass as bass  # noqa: F401  (kept for parity with bacc deps)
import concourse.mybir as mybir
import concourse.tile as tile
from concourse.bass_utils import run_bass_kernel_spmd

N, D, C = 600000, 32, 10
EPS = 0.5
NCORES = 8
CAP = 61440                      # padded rows per class block (~60000 + 6 sigma)
ROWS_PER_CORE = C * CAP // NCORES        # 76800
CHUNK_TILES = 60                 # 128-sample matmul tiles per chunk
CHUNK_ROWS = 128 * CHUNK_TILES           # 7680
CHUNKS_PER_CORE = ROWS_PER_CORE // CHUNK_ROWS    # 10
CHUNKS_PER_CLASS = CAP // CHUNK_ROWS             # 8
GROUPS = CHUNK_TILES // 4        # 15 four-tile matmul groups per chunk
SPLIT = 32                       # tiles 0..31 arrive on the sync queue, rest on scalar

_cache = {}


def _build_program():
    nc = bacc.Bacc(None)
    f16 = mybir.dt.float16
    f32 = mybir.dt.float32
    z_dram = nc.dram_tensor("Z", [ROWS_PER_CORE, D], f16, kind="ExternalInput")
    out_dram = nc.dram_tensor(
        "grams", [128, CHUNKS_PER_CORE * D], f32, kind="ExternalOutput"
    )

    with tile.TileContext(nc) as tc:
        with (
            tc.tile_pool(name="z", bufs=CHUNKS_PER_CORE) as z_pool,
            tc.tile_pool(name="outp", bufs=1) as out_pool,
            tc.tile_pool(name="psum", bufs=1, space="PSUM") as psum_pool,
        ):
            acc = psum_pool.tile([128, CHUNKS_PER_CORE * 128], f32)

            zv = z_dram.rearrange("(c p t) d -> c p (t d)", p=128, t=CHUNK_TILES)

            # Every chunk DMA is issued up front, split between the two
            # hardware DGE queues (Sync and Scalar engines) so both pull
            # from HBM at once; the whole shard stays resident in SBUF.
            z_tiles = []
            for c in range(CHUNKS_PER_CORE):
                z_sb = z_pool.tile([128, CHUNK_TILES * D], f16, tag="z")
                nc.sync.dma_start(z_sb[:, : SPLIT * D], zv[c][:, : SPLIT * D])
                nc.scalar.dma_start(z_sb[:, SPLIT * D :], zv[c][:, SPLIT * D :])
                z_tiles.append(z_sb)

            # As soon as a chunk's 15th matmul retires, its four diagonal
            # 32x32 blocks are copied out to SBUF (two bands on the DVE,
            # two on the Activation engine), overlapping the remaining
            # chunks' matmuls; only chunk 9's extraction trails the PE.
            out_sb = out_pool.tile([128, CHUNKS_PER_CORE * D], f32)
            for c in range(CHUNKS_PER_CORE):
                z_sb = z_tiles[c]
                for g in range(GROUPS):
                    zg = z_sb[:, g * 4 * D : (g + 1) * 4 * D]
                    nc.tensor.matmul(
                        acc[:, c * 128 : (c + 1) * 128],
                        zg,
                        zg,
                        start=(g == 0),
                        stop=(g == GROUPS - 1),
                    )
                for b in range(4):
                    src = acc[b * D : (b + 1) * D, c * 128 + b * D : c * 128 + (b + 1) * D]
                    dst = out_sb[b * D : (b + 1) * D, c * D : (c + 1) * D]
                    if b < 2:
                        nc.vector.tensor_copy(dst, src)
                    else:
                        nc.scalar.mul(dst, src, 1.0)
            nc.sync.dma_start(out_dram[:], out_sb[:])

    nc.compile()
    return nc


def kernel(Z: np.ndarray, labels: np.ndarray) -> np.ndarray:
    Z = np.asarray(Z, dtype=np.float32)
    labels = np.asarray(labels, dtype=np.int32)

    if "nc" not in _cache:
        _cache["nc"] = _build_program()
    nc = _cache["nc"]

    counts = np.bincount(labels, minlength=C)
    order = np.argsort(labels, kind="stable")

    Zp = np.zeros([C * CAP, D], np.float16)
    host_extra = np.zeros([C, D, D], np.float64)
    off = 0
    for j in range(C):
        cnt = int(counts[j])
        take = min(cnt, CAP)
        Zp[j * CAP : j * CAP + take] = Z[order[off : off + take]]
        if cnt > CAP:
            extra = Z[order[off + take : off + cnt]].astype(np.float64)
            host_extra[j] = extra.T @ extra
        off += cnt

    in_maps = [
        {"Z": Zp[k * ROWS_PER_CORE : (k + 1) * ROWS_PER_CORE]}
        for k in range(NCORES)
    ]

    res = run_bass_kernel_spmd(nc, in_maps, core_ids=list(range(NCORES)))
    _cache["last_results"] = res

    gj = host_extra.copy()
    for k, r in enumerate(res.results):
        # [128, 320] -> bands summed -> [32, 10, 32] per-chunk partials
        g = r["grams"].astype(np.float64).reshape(4, D, CHUNKS_PER_CORE, D).sum(axis=0)
        for c in range(CHUNKS_PER_CORE):
            gj[(k * CHUNKS_PER_CORE + c) // CHUNKS_PER_CLASS] += g[:, c, :]

    g_all = gj.sum(axis=0)
    tr_pi = counts.astype(np.float64)

    nf, df = float(N), float(D)
    eye = np.eye(D)
    loss_r = 0.5 * np.linalg.slogdet(eye + (df / (nf * EPS)) * g_all)[1]
    loss_rc = 0.0
    for j in range(C):
        ld = np.linalg.slogdet(eye + (df / (tr_pi[j] * EPS)) * gj[j])[1]
        loss_rc += (tr_pi[j] / (2.0 * nf)) * ld
    loss_obj = loss_r - loss_rc
    return np.asarray([-loss_obj, loss_r, loss_rc], dtype=np.float32)


# revision 6
# speedup vs baseline: 1.5391x; 1.5391x over previous
"""MCR2 loss kernel for 8 Trainium2 NeuronCores.

Class-sorted data-parallel sharding: the host permutes samples so each
class occupies a contiguous, zero-padded block of CAP rows (one-hot
masking then costs nothing on device).  Each core streams its 76800-row
shard of the permuted Z once as fp16.  Tensor work is batched four
128-sample tiles per matmul: stationary = moving = a [128, 128] column
block of four adjacent tiles, accumulated into one [128, 128] PSUM
region per 7680-row chunk.  The four diagonal 32x32 blocks of each
region are the per-tile Grams (off-diagonal cross blocks accumulate
into distinct PSUM addresses and are never read).  Every chunk lies
inside one class block, so chunk-Grams sum to class-Grams on the host,
where the 32x32 logdets are evaluated in float64.
"""

import sys

sys.path.insert(0, "/opt/trn_rl_repo")

import numpy as np

import concourse.bacc as bacc
import concourse.bass as bass  # noqa: F401  (kept for parity with bacc deps)
import concourse.mybir as mybir
import concourse.tile as tile
from concourse.bass_utils import run_bass_kernel_spmd

N, D, C = 600000, 32, 10
EPS = 0.5
NCORES = 8
CAP = 61440                      # padded rows per class block (~60000 + 6 sigma)
ROWS_PER_CORE = C * CAP // NCORES        # 76800
CHUNK_TILES = 60                 # 128-sample matmul tiles per chunk
CHUNK_ROWS = 128 * CHUNK_TILES           # 7680
CHUNKS_PER_CORE = ROWS_PER_CORE // CHUNK_ROWS    # 10
CHUNKS_PER_CLASS = CAP // CHUNK_ROWS             # 8
GROUPS = CHUNK_TILES // 4        # 15 four-tile matmul groups per chunk
SPLIT = 32                       # tiles 0..31 arrive on the sync queue, rest on scalar

_cache = {}


def _build_program():
    nc = bacc.Bacc(None)
    f16 = mybir.dt.float16
    f32 = mybir.dt.float32
    z_dram = nc.dram_tensor("Z", [ROWS_PER_CORE, D], f16, kind="ExternalInput")
    out_dram = nc.dram_tensor(
        "grams", [128, CHUNKS_PER_CORE * D], f32, kind="ExternalOutput"
    )

    with tile.TileContext(nc) as tc:
        with (
            tc.tile_pool(name="z", bufs=CHUNKS_PER_CORE) as z_pool,
            tc.tile_pool(name="outp", bufs=1) as out_pool,
            tc.tile_pool(name="psum", bufs=6, space="PSUM") as psum_pool,
        ):
            zv = z_dram.rearrange("(c p t) d -> c p (t d)", p=128, t=CHUNK_TILES)

            # Every chunk DMA is issued up front, split between the two
            # hardware DGE queues (Sync and Scalar engines) so both pull
            # from HBM at once; the whole shard stays resident in SBUF.
            z_tiles = []
            for c in range(CHUNKS_PER_CORE):
                z_sb = z_pool.tile([128, CHUNK_TILES * D], f16, tag="z")
                nc.sync.dma_start(z_sb[:, : SPLIT * D], zv[c][:, : SPLIT * D])
                nc.scalar.dma_start(z_sb[:, SPLIT * D :], zv[c][:, SPLIT * D :])
                z_tiles.append(z_sb)

            # Each chunk accumulates into its own PSUM bank (rotating pool,
            # six deep so a fresh chunk's start=True never waits on a recent
            # extraction).  As soon as a chunk's 15th matmul retires, its
            # four diagonal 32x32 blocks are copied to SBUF on the idle DVE,
            # overlapping the remaining chunks' matmuls.
            out_sb = out_pool.tile([128, CHUNKS_PER_CORE * D], f32)
            for c in range(CHUNKS_PER_CORE):
                z_sb = z_tiles[c]
                acc = psum_pool.tile([128, 128], f32, tag="acc")
                for g in range(GROUPS):
                    zg = z_sb[:, g * 4 * D : (g + 1) * 4 * D]
                    nc.tensor.matmul(
                        acc[:],
                        zg,
                        zg,
                        start=(g == 0),
                        stop=(g == GROUPS - 1),
                    )
                for b in range(4):
                    nc.vector.tensor_copy(
                        out_sb[b * D : (b + 1) * D, c * D : (c + 1) * D],
                        acc[b * D : (b + 1) * D, b * D : (b + 1) * D],
                    )
            nc.sync.dma_start(out_dram[:], out_sb[:])

    nc.compile()
    return nc


def kernel(Z: np.ndarray, labels: np.ndarray) -> np.ndarray:
    Z = np.asarray(Z, dtype=np.float32)
    labels = np.asarray(labels, dtype=np.int32)

    if "nc" not in _cache:
        _cache["nc"] = _build_program()
    nc = _cache["nc"]

    counts = np.bincount(labels, minlength=C)
    order = np.argsort(labels, kind="stable")

    Zp = np.zeros([C * CAP, D], np.float16)
    host_extra = np.zeros([C, D, D], np.float64)
    off = 0
    for j in range(C):
        cnt = int(counts[j])
        take = min(cnt, CAP)
        Zp[j * CAP : j * CAP + take] = Z[order[off : off + take]]
        if cnt > CAP:
            extra = Z[order[off + take : off + cnt]].astype(np.float64)
            host_extra[j] = extra.T @ extra
        off += cnt

    in_maps = [
        {"Z": Zp[k * ROWS_PER_CORE : (k + 1) * ROWS_PER_CORE]}
        for k in range(NCORES)
    ]

    res = run_bass_kernel_spmd(nc, in_maps, core_ids=list(range(NCORES)))
    _cache["last_results"] = res

    gj = host_extra.copy()
    for k, r in enumerate(res.results):
        # [128, 320] -> bands summed -> [32, 10, 32] per-chunk partials
        g = r["grams"].astype(np.float64).reshape(4, D, CHUNKS_PER_CORE, D).sum(axis=0)
        for c in range(CHUNKS_PER_CORE):
            gj[(k * CHUNKS_PER_CORE + c) // CHUNKS_PER_CLASS] += g[:, c, :]

    g_all = gj.sum(axis=0)
    tr_pi = counts.astype(np.float64)

    nf, df = float(N), float(D)
    eye = np.eye(D)
    loss_r = 0.5 * np.linalg.slogdet(eye + (df / (nf * EPS)) * g_all)[1]
    loss_rc = 0.0
    for j in range(C):
        ld = np.linalg.slogdet(eye + (df / (tr_pi[j] * EPS)) * gj[j])[1]
        loss_rc += (tr_pi[j] / (2.0 * nf)) * ld
    loss_obj = loss_r - loss_rc
    return np.asarray([-loss_obj, loss_r, loss_rc], dtype=np.float32)


# revision 8
# speedup vs baseline: 1.7136x; 1.1134x over previous
"""MCR2 loss kernel for 8 Trainium2 NeuronCores.

Class-sorted data-parallel sharding: the host permutes samples so each
class occupies a contiguous, zero-padded block of CAP rows (one-hot
masking then costs nothing on device).  Each core streams its 76800-row
shard of the permuted Z once as fp16.  Tensor work is batched four
128-sample tiles per matmul: stationary = moving = a [128, 128] column
block of four adjacent tiles, accumulated into one [128, 128] PSUM
region per 7680-row chunk.  The four diagonal 32x32 blocks of each
region are the per-tile Grams (off-diagonal cross blocks accumulate
into distinct PSUM addresses and are never read).  Every chunk lies
inside one class block, so chunk-Grams sum to class-Grams on the host,
where the 32x32 logdets are evaluated in float64.
"""

import sys

sys.path.insert(0, "/opt/trn_rl_repo")

import numpy as np

import concourse.bacc as bacc
import concourse.bass as bass  # noqa: F401  (kept for parity with bacc deps)
import concourse.mybir as mybir
import concourse.tile as tile
from concourse.bass_utils import run_bass_kernel_spmd

N, D, C = 600000, 32, 10
EPS = 0.5
NCORES = 8
CAP = 61440                      # padded rows per class block (~60000 + 6 sigma)
ROWS_PER_CORE = C * CAP // NCORES        # 76800
CHUNK_TILES = 60                 # 128-sample matmul tiles per chunk
CHUNK_ROWS = 128 * CHUNK_TILES           # 7680
CHUNKS_PER_CORE = ROWS_PER_CORE // CHUNK_ROWS    # 10
CHUNKS_PER_CLASS = CAP // CHUNK_ROWS             # 8
GROUPS = CHUNK_TILES // 4        # 15 four-tile matmul groups per chunk
SPLIT = 32                       # tiles 0..31 arrive on the sync queue, rest on scalar

_cache = {}


def _build_program():
    nc = bacc.Bacc(None)
    f16 = mybir.dt.float16
    f32 = mybir.dt.float32
    z_dram = nc.dram_tensor("Z", [ROWS_PER_CORE, D], f16, kind="ExternalInput")
    out_dram = nc.dram_tensor(
        "grams", [128, CHUNKS_PER_CORE * D], f32, kind="ExternalOutput"
    )

    with tile.TileContext(nc) as tc:
        with (
            tc.tile_pool(name="z", bufs=CHUNKS_PER_CORE) as z_pool,
            tc.tile_pool(name="outp", bufs=1) as out_pool,
            tc.tile_pool(name="psum", bufs=8, space="PSUM") as psum_pool,
        ):
            zv = z_dram.rearrange("(c p t) d -> c p (t d)", p=128, t=CHUNK_TILES)

            # Every chunk DMA is issued up front, alternating between the two
            # hardware DGE queues (Sync and Scalar engines) so both pull from
            # HBM at once; the whole shard stays resident in SBUF.  Chunk 0
            # alone is split across both queues so the PE starts sooner.
            z_tiles = []
            for c in range(CHUNKS_PER_CORE):
                z_sb = z_pool.tile([128, CHUNK_TILES * D], f16, tag="z")
                if c == 0:
                    nc.sync.dma_start(z_sb[:, : SPLIT * D], zv[c][:, : SPLIT * D])
                    nc.scalar.dma_start(z_sb[:, SPLIT * D :], zv[c][:, SPLIT * D :])
                else:
                    eng = nc.sync if c % 2 else nc.scalar
                    eng.dma_start(z_sb[:], zv[c])
                z_tiles.append(z_sb)

            # Each chunk accumulates into its own PSUM bank (rotating pool,
            # eight deep so a fresh chunk's start=True never waits on a recent
            # extraction).  As soon as a chunk's 15th matmul retires, its
            # four diagonal 32x32 blocks are copied to SBUF on the idle DVE,
            # overlapping the remaining chunks' matmuls.
            out_sb = out_pool.tile([128, CHUNKS_PER_CORE * D], f32)
            for c in range(CHUNKS_PER_CORE):
                z_sb = z_tiles[c]
                acc = psum_pool.tile([128, 128], f32, tag="acc")
                for g in range(GROUPS):
                    zg = z_sb[:, g * 4 * D : (g + 1) * 4 * D]
                    nc.tensor.matmul(
                        acc[:],
                        zg,
                        zg,
                        start=(g == 0),
                        stop=(g == GROUPS - 1),
                    )
                for b in range(4):
                    nc.vector.tensor_copy(
                        out_sb[b * D : (b + 1) * D, c * D : (c + 1) * D],
                        acc[b * D : (b + 1) * D, b * D : (b + 1) * D],
                    )
            nc.sync.dma_start(out_dram[:], out_sb[:])

    nc.compile()
    return nc


def kernel(Z: np.ndarray, labels: np.ndarray) -> np.ndarray:
    Z = np.asarray(Z, dtype=np.float32)
    labels = np.asarray(labels, dtype=np.int32)

    if "nc" not in _cache:
        _cache["nc"] = _build_program()
    nc = _cache["nc"]

    counts = np.bincount(labels, minlength=C)
    order = np.argsort(labels, kind="stable")

    Zp = np.zeros([C * CAP, D], np.float16)
    host_extra = np.zeros([C, D, D], np.float64)
    off = 0
    for j in range(C):
        cnt = int(counts[j])
        take = min(cnt, CAP)
        Zp[j * CAP : j * CAP + take] = Z[order[off : off + take]]
        if cnt > CAP:
            extra = Z[order[off + take : off + cnt]].astype(np.float64)
            host_extra[j] = extra.T @ extra
        off += cnt

    in_maps = [
        {"Z": Zp[k * ROWS_PER_CORE : (k + 1) * ROWS_PER_CORE]}
        for k in range(NCORES)
    ]

    res = run_bass_kernel_spmd(nc, in_maps, core_ids=list(range(NCORES)))
    _cache["last_results"] = res

    gj = host_extra.copy()
    for k, r in enumerate(res.results):
        # [128, 320] -> bands summed -> [32, 10, 32] per-chunk partials
        g = r["grams"].astype(np.float64).reshape(4, D, CHUNKS_PER_CORE, D).sum(axis=0)
        for c in range(CHUNKS_PER_CORE):
            gj[(k * CHUNKS_PER_CORE + c) // CHUNKS_PER_CLASS] += g[:, c, :]

    g_all = gj.sum(axis=0)
    tr_pi = counts.astype(np.float64)

    nf, df = float(N), float(D)
    eye = np.eye(D)
    loss_r = 0.5 * np.linalg.slogdet(eye + (df / (nf * EPS)) * g_all)[1]
    loss_rc = 0.0
    for j in range(C):
        ld = np.linalg.slogdet(eye + (df / (tr_pi[j] * EPS)) * gj[j])[1]
        loss_rc += (tr_pi[j] / (2.0 * nf)) * ld
    loss_obj = loss_r - loss_rc
    return np.asarray([-loss_obj, loss_r, loss_rc], dtype=np.float32)
